# revision 1
# baseline (speedup 1.0000x reference)
"""Trainium2 Bass kernel for dual channel-attention block (nn_Attention_85985245266248).

Strategy:
  - Shard spatially: 256 rows -> 8 cores x 32 rows, each core's input shard
    carries a 1-row halo (zero at global edges) and 1-col zero padding.
  - conv1x1 + depthwise3x3 folded into a full 3x3 conv (rank-1 weights),
    executed as 9 PSUM-accumulated matmuls per tile on the PE.
  - Pass A computes q,k in [px, ch] layout (input stationary, weights moving)
    so the c-x-c Gram matrices q@k^T and the L2 norms come straight off the
    PE with pixel-contraction; partial Grams are AllReduce'd across cores.
  - Pass B computes v in [ch, px] layout (weights stationary).
  - Softmax + norm scaling on DVE/ACT (tiny 96x96 tensors).
  - Output projection po/concat folded on host into P_c/P_t; final output is
    two accumulated matmuls per pixel chunk: out = M_cT^T @ v_t + M_tT^T @ v_c + b.
All heavy matmuls run in bf16 (fp32 accumulate in PSUM).
"""
import os
import sys
import numpy as np

sys.path.insert(0, "/opt/trn_rl_repo")

B = 2
D = 96
H = 256
W = 256
HEADS = 3
NC = 8
RPC = H // NC          # rows per core = 32
HR = RPC + 2           # halo rows = 34
PW = W + 2             # padded width = 258
PXT = 128              # pass-A pixel tile (half row)
NT_A = RPC * W // PXT  # pass-A tiles per batch per tensor = 64
CHK = 512              # pass-B / final chunk = 2 rows
NCHK = RPC * W // CHK  # 16

_CACHE = {}


def _fold3x3(w1, dw):
    """w1:[O,C], dw:[O,1,3,3] -> [9, C, O] rhs-layout folded weights."""
    O, C = w1.shape
    out = np.zeros((9, C, O), np.float32)
    for t in range(9):
        dy, dx = t // 3, t % 3
        out[t] = (dw[:, 0, dy, dx][:, None] * w1).T
    return out


def _bf16(a):
    import ml_dtypes
    return np.asarray(a, np.float32).astype(ml_dtypes.bfloat16)


def _build(nc_mod):
    """Build the Bass program (uses modules passed in)."""
    bass, bacc, tile, mybir = nc_mod
    f32 = mybir.dt.float32
    bf16 = mybir.dt.bfloat16

    nc = bacc.Bacc("TRN2", target_bir_lowering=False, debug=False, num_devices=NC)

    # I/O: per-core shards (bf16 inputs pre-padded on host)
    x_hi = nc.dram_tensor("x_hi", [B, D, HR, PW], bf16, kind="ExternalInput")
    x_lo = nc.dram_tensor("x_lo", [B, D, HR, PW], bf16, kind="ExternalInput")
    wqk_hi = nc.dram_tensor("wqk_hi", [D, 9, 2 * D], bf16, kind="ExternalInput")
    wqk_lo = nc.dram_tensor("wqk_lo", [D, 9, 2 * D], bf16, kind="ExternalInput")
    wv_hi = nc.dram_tensor("wv_hi", [D, 9, D], bf16, kind="ExternalInput")
    wv_lo = nc.dram_tensor("wv_lo", [D, 9, D], bf16, kind="ExternalInput")
    pct = nc.dram_tensor("pct", [D, D], bf16, kind="ExternalInput")
    ptt = nc.dram_tensor("ptt", [D, D], bf16, kind="ExternalInput")
    ident = nc.dram_tensor("ident", [D, D], f32, kind="ExternalInput")
    tempvec = nc.dram_tensor("tempvec", [D, 1], f32, kind="ExternalInput")
    biasvec = nc.dram_tensor("biasvec", [D, 1], f32, kind="ExternalInput")
    out_ext = nc.dram_tensor("out", [B, D, RPC, W], f32, kind="ExternalOutput")

    NG = 6  # grams per batch: G1, G2, Sqc, Skc, Sqt, Skt

    with tile.TileContext(nc) as tc:
        with (
            tc.tile_pool(name="consts", bufs=1) as cpool,
            tc.tile_pool(name="xres", bufs=2) as xpool,
            tc.tile_pool(name="vres", bufs=1) as vpool,
            tc.tile_pool(name="qk", bufs=4) as qkpool,
            tc.tile_pool(name="work_ps", bufs=3, space="PSUM") as wps,
            tc.tile_pool(name="gram_ps", bufs=1, space="PSUM") as gps,
            tc.tile_pool(name="small", bufs=1) as spool,
            tc.tile_pool(name="dram", bufs=1, space="DRAM") as dpool,
        ):
            # ---- load constants ----
            wqk_hi_sb = cpool.tile([D, 9, 2 * D], bf16, tag="wqkh")
            wqk_lo_sb = cpool.tile([D, 9, 2 * D], bf16, tag="wqkl")
            wv_hi_sb = cpool.tile([D, 9, D], bf16, tag="wvh")
            wv_lo_sb = cpool.tile([D, 9, D], bf16, tag="wvl")
            pct_sb = cpool.tile([D, D], bf16, tag="pct")
            ptt_sb = cpool.tile([D, D], bf16, tag="ptt")
            ident_sb = cpool.tile([D, D], f32, tag="ident")
            tempv_sb = cpool.tile([D, 1], f32, tag="tempv")
            biasv_sb = cpool.tile([D, 1], f32, tag="biasv")
            nc.sync.dma_start(out=wqk_hi_sb[:], in_=wqk_hi[:])
            nc.sync.dma_start(out=wqk_lo_sb[:], in_=wqk_lo[:])
            nc.sync.dma_start(out=wv_hi_sb[:], in_=wv_hi[:])
            nc.sync.dma_start(out=wv_lo_sb[:], in_=wv_lo[:])
            nc.sync.dma_start(out=pct_sb[:], in_=pct[:])
            nc.sync.dma_start(out=ptt_sb[:], in_=ptt[:])
            nc.sync.dma_start(out=ident_sb[:], in_=ident[:])
            nc.sync.dma_start(out=tempv_sb[:], in_=tempvec[:])
            nc.sync.dma_start(out=biasv_sb[:], in_=biasvec[:])

            # gram accumulation targets and per-batch v stores
            gram_cat = spool.tile([D, B * NG * D], f32, tag="gramcat")
            v_sb = {}   # (b, 'hi'/'lo') -> [D, RPC*W] bf16
            for b in range(B):
                for s in ("hi", "lo"):
                    v_sb[(b, s)] = vpool.tile([D, RPC * W], bf16,
                                              tag=f"v{b}{s}", name=f"v{b}{s}")

            xt = {}
            for b in range(B):
                # ---- load this batch's input shards ----
                xh = xpool.tile([D, HR, PW], bf16, tag="xh")
                xl = xpool.tile([D, HR, PW], bf16, tag="xl")
                nc.sync.dma_start(out=xh[:], in_=x_hi[b])
                nc.sync.dma_start(out=xl[:], in_=x_lo[b])
                xt[(b, "hi")] = xh
                xt[(b, "lo")] = xl
                del xh, xl

                # ---- pass A: q,k in [px, ch] + Gram/norm accumulation ----
                # paired layout sbp[:, g, :]: g=0 -> [q_c | k_t], g=1 -> [k_c | q_t]
                gA = gps.tile([D, 2 * D], f32, tag="gA", name=f"gA{b}")  # [Sqc | G1]
                gB = gps.tile([D, 2 * D], f32, tag="gB", name=f"gB{b}")  # [G2 | Sqt]
                gC = gps.tile([D, D], f32, tag="gC", name=f"gC{b}")      # Skt
                gD = gps.tile([D, D], f32, tag="gD", name=f"gD{b}")      # Skc

                def grams(sbp, first, last):
                    nc.tensor.matmul(gA[:], sbp[:, 0, 0:D], sbp[:, 0, :],
                                     start=first, stop=last)
                    nc.tensor.matmul(gB[:], sbp[:, 1, D:2 * D], sbp[:, 1, :],
                                     start=first, stop=last)
                    nc.tensor.matmul(gC[:], sbp[:, 0, D:2 * D], sbp[:, 0, D:2 * D],
                                     start=first, stop=last)
                    nc.tensor.matmul(gD[:], sbp[:, 1, 0:D], sbp[:, 1, 0:D],
                                     start=first, stop=last)

                prev = None
                for it in range(NT_A):
                    r = (it * PXT) // W          # output row 0..31
                    j = (it * PXT) % W           # 0 or 128
                    sbp = qkpool.tile([PXT, 2, 2 * D], bf16, tag="qksb")
                    for gi, (s, wsb) in enumerate((("hi", wqk_hi_sb),
                                                   ("lo", wqk_lo_sb))):
                        ps = wps.tile([PXT, 2 * D], f32, tag="apsum")
                        xs = xt[(b, s)]
                        for t in range(9):
                            dy, dx = t // 3, t % 3
                            lhsT = xs[:, r + dy, j + dx:j + dx + PXT]
                            nc.tensor.matmul(ps[:], lhsT, wsb[:, t, :],
                                             start=(t == 0), stop=(t == 8))
                        # hi [q_c|k_c] -> cols {0:96, 192:288}; lo [k_t|q_t] -> {96:192, 288:384}
                        nc.vector.tensor_copy(sbp[:, :, gi * D:(gi + 1) * D], ps[:])
                    if prev is not None:
                        grams(prev, prev_first, False)
                    prev_first = prev is None
                    prev = sbp
                grams(prev, False, True)

                for k, src in (("G1", gA[:, D:2 * D]), ("G2", gB[:, 0:D]),
                               ("Sqc", gA[:, 0:D]), ("Skc", gD[:]),
                               ("Sqt", gB[:, D:2 * D]), ("Skt", gC[:])):
                    gi = ("G1", "G2", "Sqc", "Skc", "Sqt", "Skt").index(k)
                    off = (b * NG + gi) * D
                    nc.vector.tensor_copy(gram_cat[:, off:off + D], src)

                # ---- pass B: v in [ch, px] ----
                for s, wsb in (("hi", wv_hi_sb), ("lo", wv_lo_sb)):
                    xs = xt[(b, s)]
                    for ck in range(NCHK):
                        r = ck * 2
                        ps = wps.tile([D, CHK], f32, tag="apsum")
                        for t in range(9):
                            dy, dx = t // 3, t % 3
                            rhs = xs[:, r + dy:r + dy + 2, dx:dx + W]
                            nc.tensor.matmul(ps[:], wsb[:, t, :], rhs,
                                             start=(t == 0), stop=(t == 8))
                        nc.vector.tensor_copy(
                            v_sb[(b, s)][:, ck * CHK:(ck + 1) * CHK], ps[:])

            # ---- AllReduce partial grams across the 8 cores ----
            ar_in = dpool.tile([D, B * NG * D], f32, tag="arin")
            ar_out = dpool.tile([D, B * NG * D], f32, tag="arout")
            nc.gpsimd.dma_start(out=ar_in[:], in_=gram_cat[:])
            nc.gpsimd.collective_compute(
                "AllReduce",
                mybir.AluOpType.add,
                replica_groups=[list(range(NC))],
                ins=[ar_in.opt()],
                outs=[ar_out.opt()],
            )
            gram_red = spool.tile([D, B * NG * D], f32, tag="gramred")
            nc.gpsimd.dma_start(out=gram_red[:], in_=ar_out[:])

            # ---- post-AR small compute per batch ----
            mt = {}  # (b, 'c'/'t') -> M^T tile [D, D] bf16
            for b in range(B):
                def gslice(gi):
                    off = (b * NG + gi) * D
                    return gram_red[:, off:off + D]
                G1, G2, Sqc, Skc, Sqt, Skt = [gslice(i) for i in range(NG)]

                rcol = {}
                for nm, S in (("qc", Sqc), ("kc", Skc), ("qt", Sqt), ("kt", Skt)):
                    tmp = spool.tile([D, D], f32, tag="dtmp")
                    nc.vector.tensor_tensor(out=tmp[:], in0=S, in1=ident_sb[:],
                                            op=mybir.AluOpType.mult)
                    dg = spool.tile([D, 1], f32, tag=f"d{nm}{b}")
                    nc.vector.tensor_reduce(out=dg[:], in_=tmp[:],
                                            axis=mybir.AxisListType.X,
                                            op=mybir.AluOpType.add)
                    sq = spool.tile([D, 1], f32, tag=f"sq{nm}{b}")
                    nc.scalar.sqrt(sq[:], dg[:])
                    rc = spool.tile([D, 1], f32, tag=f"rc{nm}{b}")
                    nc.vector.reciprocal(rc[:], sq[:])
                    rcol[nm] = rc
                # fold temperature into rq
                for nm in ("qc", "qt"):
                    nc.vector.tensor_tensor(out=rcol[nm][:], in0=rcol[nm][:],
                                            in1=tempv_sb[:],
                                            op=mybir.AluOpType.mult)

                # row-vector 1/||k|| via partition reduce of (S*I)
                rrow = {}
                for nm, S in (("kt", Skt), ("kc", Skc)):
                    tmp = spool.tile([D, D], f32, tag="dtmp")
                    nc.vector.tensor_tensor(out=tmp[:], in0=S, in1=ident_sb[:],
                                            op=mybir.AluOpType.mult)
                    drow = spool.tile([1, D], f32, tag=f"dr{nm}{b}")
                    nc.gpsimd.tensor_reduce(out=drow[:], in_=tmp[:],
                                            axis=mybir.AxisListType.C,
                                            op=mybir.AluOpType.add)
                    sqr = spool.tile([1, D], f32, tag=f"sqr{nm}{b}")
                    nc.scalar.sqrt(sqr[:], drow[:])
                    rr = spool.tile([1, D], f32, tag=f"rr{nm}{b}")
                    nc.vector.reciprocal(rr[:], sqr[:])
                    rb = spool.tile([D, D], f32, tag=f"rb{nm}{b}")
                    nc.gpsimd.partition_broadcast(rb[:], rr[:])
                    rrow[nm] = rb

                for attn_nm, G, rq, rkb, psb in (
                        ("c", G1, rcol["qc"], rrow["kt"], pct_sb),
                        ("t", G2, rcol["qt"], rrow["kc"], ptt_sb)):
                    L = spool.tile([D, D], f32, tag=f"L{attn_nm}{b}")
                    nc.vector.tensor_scalar(out=L[:], in0=G, scalar1=rq[:],
                                            scalar2=None,
                                            op0=mybir.AluOpType.mult)
                    nc.vector.tensor_tensor(out=L[:], in0=L[:], in1=rkb[:],
                                            op=mybir.AluOpType.mult)
                    A = spool.tile([D, D], bf16, tag=f"A{attn_nm}{b}")
                    nc.vector.memset(A[:], 0.0)
                    for h in range(HEADS):
                        p0 = 32 * h
                        blk = L[p0:p0 + 32, p0:p0 + 32]
                        nmax = spool.tile([32, 1], f32, tag=f"nm{attn_nm}{b}{h}")
                        nc.vector.tensor_reduce(out=nmax[:], in_=blk,
                                                axis=mybir.AxisListType.X,
                                                op=mybir.AluOpType.max,
                                                negate=True)
                        e = spool.tile([32, 32], f32, tag=f"e{attn_nm}{b}{h}")
                        nc.scalar.activation(e[:], blk,
                                             mybir.ActivationFunctionType.Exp,
                                             bias=nmax[:], scale=1.0)
                        ssum = spool.tile([32, 1], f32, tag=f"ss{attn_nm}{b}{h}")
                        nc.vector.tensor_reduce(out=ssum[:], in_=e[:],
                                                axis=mybir.AxisListType.X,
                                                op=mybir.AluOpType.add)
                        rs = spool.tile([32, 1], f32, tag=f"rs{attn_nm}{b}{h}")
                        nc.vector.reciprocal(rs[:], ssum[:])
                        nc.vector.tensor_scalar(out=A[p0:p0 + 32, p0:p0 + 32],
                                                in0=e[:], scalar1=rs[:],
                                                scalar2=None,
                                                op0=mybir.AluOpType.mult)
                    # M^T = A(lhsT) . P^T  -> [d, o]
                    mps = wps.tile([D, D], f32, tag="apsum")
                    nc.tensor.matmul(mps[:], A[:], psb[:], start=True, stop=True)
                    msb = spool.tile([D, D], bf16, tag=f"m{attn_nm}{b}")
                    nc.vector.tensor_copy(msb[:], mps[:])
                    mt[(b, attn_nm)] = msb

            # ---- final: out = M_cT^T @ v_t + M_tT^T @ v_c + bias ----
            for b in range(B):
                for ck in range(NCHK):
                    ps = wps.tile([D, CHK], f32, tag="apsum")
                    sl = slice(ck * CHK, (ck + 1) * CHK)
                    nc.tensor.matmul(ps[:], mt[(b, "c")][:], v_sb[(b, "lo")][:, sl],
                                     start=True, stop=False)
                    nc.tensor.matmul(ps[:], mt[(b, "t")][:], v_sb[(b, "hi")][:, sl],
                                     start=False, stop=True)
                    osb = qkpool.tile([D, CHK], f32, tag="osb")
                    nc.scalar.activation(osb[:], ps[:],
                                         mybir.ActivationFunctionType.Identity,
                                         bias=biasv_sb[:], scale=1.0)
                    r = ck * 2
                    nc.sync.dma_start(out=out_ext[b, :, r:r + 2, :], in_=osb[:])

    nc.compile()
    return nc


def _get_nc():
    if "nc" not in _CACHE:
        from concourse import bass, bacc, tile, mybir
        _CACHE["mods"] = (bass, bacc, tile, mybir)
        _CACHE["nc"] = _build(_CACHE["mods"])
    return _CACHE["nc"]


def _prep_inputs(low, high, temperature, qc_w, qdw_c_w, kvc_w, kvdw_c_w,
                 qt_w, qdw_t_w, kvt_w, kvdw_t_w, po_c_w, po_t_w,
                 concat_w, concat_b):
    """Host-side weight folding + input shard/pad/cast. Returns in_maps."""
    W3 = {
        "q_hi": _fold3x3(qc_w, qdw_c_w),
        "k_hi": _fold3x3(kvc_w[:96], kvdw_c_w[:96]),
        "v_hi": _fold3x3(kvc_w[96:], kvdw_c_w[96:]),
        "q_lo": _fold3x3(qt_w, qdw_t_w),
        "k_lo": _fold3x3(kvt_w[:96], kvdw_t_w[:96]),
        "v_lo": _fold3x3(kvt_w[96:], kvdw_t_w[96:]),
    }
    wqk_hi = _bf16(np.concatenate([W3["q_hi"], W3["k_hi"]], axis=2))  # [9,96,192]
    wqk_lo = _bf16(np.concatenate([W3["k_lo"], W3["q_lo"]], axis=2))
    wv_hi = _bf16(W3["v_hi"])
    wv_lo = _bf16(W3["v_lo"])
    # device layout [D(ci), 9, O]
    wqk_hi = np.ascontiguousarray(wqk_hi.transpose(1, 0, 2))
    wqk_lo = np.ascontiguousarray(wqk_lo.transpose(1, 0, 2))
    wv_hi = np.ascontiguousarray(wv_hi.transpose(1, 0, 2))
    wv_lo = np.ascontiguousarray(wv_lo.transpose(1, 0, 2))
    P_c = concat_w[:, :96] @ po_c_w
    P_t = concat_w[:, 96:] @ po_t_w
    pct = _bf16(P_c.T)
    ptt = _bf16(P_t.T)
    ident = np.eye(D, dtype=np.float32)
    tempv = np.repeat(np.asarray(temperature, np.float32).reshape(3), 32)[:, None]
    biasv = np.asarray(concat_b, np.float32)[:, None]

    # pad inputs: 1 col of zeros each side, 1 halo row each side of shard
    def shard(x):
        xp = np.zeros((B, D, H + 2, PW), np.float32)
        xp[:, :, 1:H + 1, 1:W + 1] = x
        sh = []
        for c in range(NC):
            r0 = c * RPC
            sh.append(_bf16(xp[:, :, r0:r0 + HR, :]))
        return sh

    lo_sh = shard(np.asarray(low, np.float32))
    hi_sh = shard(np.asarray(high, np.float32))

    in_maps = []
    for c in range(NC):
        in_maps.append({
            "x_hi": np.ascontiguousarray(hi_sh[c]),
            "x_lo": np.ascontiguousarray(lo_sh[c]),
            "wqk_hi": wqk_hi, "wqk_lo": wqk_lo,
            "wv_hi": wv_hi, "wv_lo": wv_lo,
            "pct": pct, "ptt": ptt,
            "ident": ident, "tempvec": tempv, "biasvec": biasv,
        })
    return in_maps


def run(trace=False, in_maps=None, **inputs):
    import time as _time
    from concourse.bass_utils import run_bass_kernel_spmd
    nc = _get_nc()
    if in_maps is None:
        in_maps = _prep_inputs(**inputs)
    t0 = _time.time()
    res = run_bass_kernel_spmd(nc, in_maps, list(range(NC)), trace=trace)
    res.dispatch_wall_s = _time.time() - t0
    res.in_maps = in_maps
    out = np.concatenate([res.results[c]["out"] for c in range(NC)], axis=2)
    return out.astype(np.float32), res


def kernel(**inputs):
    out, _ = run(trace=False, **inputs)
    return out



# revision 6
# speedup vs baseline: 2.5918x; 2.5918x over previous
"""Trainium2 Bass kernel for dual channel-attention block (nn_Attention_85985245266248).

Strategy:
  - Shard spatially: 256 rows -> 8 cores x 32 rows, each core's input shard
    carries a 1-row halo (zero at global edges) and 1-col zero padding.
  - Axon-tunnel traffic is the wall-clock bottleneck (~38MB/s), so all
    transfers are quantized: inputs int8 with per-(batch,channel) scales
    (dequantized on device), outputs int8 with per-(core,batch,channel)
    scales (quantized on device, round-to-nearest), weights bf16 sharded
    1/8th per core and AllGather'd on device.
  - conv1x1 + depthwise3x3 folded into a full 3x3 conv (rank-1 weights),
    executed as 9 PSUM-accumulated matmuls per tile on the PE.
  - Pass A computes q,k in [px, ch] layout (input stationary, weights moving)
    so the c-x-c Gram matrices q@k^T and the L2 norms come straight off the
    PE with pixel-contraction; partial Grams are AllReduce'd across cores.
  - Pass B computes v in [ch, px] layout (weights stationary).
  - Softmax + norm scaling on DVE/ACT (tiny 96x96 tensors).
  - Output projection po/concat folded on host into P_c/P_t; final output is
    two accumulated matmuls per pixel chunk: out = M_cT^T @ v_t + M_tT^T @ v_c + b.
All heavy matmuls run in bf16 (fp32 accumulate in PSUM).
"""
import os
import sys
import numpy as np

sys.path.insert(0, "/opt/trn_rl_repo")

B = 2
D = 96
H = 256
W = 256
HEADS = 3
NC = 8
RPC = H // NC          # rows per core = 32
HR = RPC + 2           # halo rows = 34
PW = W + 2             # padded width = 258
PXT = 128              # pass-A pixel tile (half row)
NT_A = RPC * W // PXT  # pass-A tiles per batch per tensor = 64
CHK = 512              # pass-B / final chunk = 2 rows
NCHK = RPC * W // CHK  # 16

# flat bf16 weight-gather layout: (name, elems)
WPACK = [
    ("wqk_hi", D * 9 * 2 * D),   # 165888
    ("wqk_lo", D * 9 * 2 * D),   # 165888
    ("wv_hi", D * 9 * D),        # 82944
    ("wv_lo", D * 9 * D),        # 82944
    ("pct", D * D),              # 9216
    ("ptt", D * D),              # 9216
    ("ident", D * D),            # 9216
]
WTOT = sum(n for _, n in WPACK)  # 525312
WSH = WTOT // NC                 # 65664 per core

_CACHE = {}


def _fold3x3(w1, dw):
    """w1:[O,C], dw:[O,1,3,3] -> [9, C, O] rhs-layout folded weights."""
    O, C = w1.shape
    out = np.zeros((9, C, O), np.float32)
    for t in range(9):
        dy, dx = t // 3, t % 3
        out[t] = (dw[:, 0, dy, dx][:, None] * w1).T
    return out


def _bf16(a):
    import ml_dtypes
    return np.asarray(a, np.float32).astype(ml_dtypes.bfloat16)


def _build(nc_mod):
    """Build the Bass program (uses modules passed in)."""
    bass, bacc, tile, mybir = nc_mod
    f32 = mybir.dt.float32
    bf16 = mybir.dt.bfloat16
    i8 = mybir.dt.int8

    nc = bacc.Bacc("TRN2", target_bir_lowering=False, debug=False, num_devices=NC)

    # I/O: per-core shards. Inputs int8 (pre-padded on host), weights
    # sharded bf16 (1/8th per core), outputs int8 + per-channel scales.
    x_hi = nc.dram_tensor("x_hi", [B, D, HR, PW], i8, kind="ExternalInput")
    x_lo = nc.dram_tensor("x_lo", [B, D, HR, PW], i8, kind="ExternalInput")
    sc_hi = nc.dram_tensor("sc_hi", [B, D, 1], f32, kind="ExternalInput")
    sc_lo = nc.dram_tensor("sc_lo", [B, D, 1], f32, kind="ExternalInput")
    wsh = nc.dram_tensor("wsh", [1, WSH], bf16, kind="ExternalInput")
    tempvec = nc.dram_tensor("tempvec", [D, 1], f32, kind="ExternalInput")
    biasvec = nc.dram_tensor("biasvec", [D, 1], f32, kind="ExternalInput")
    out_i8 = nc.dram_tensor("out_i8", [B, D, RPC, W], i8, kind="ExternalOutput")
    out_sc = nc.dram_tensor("out_sc", [B, D, 1], f32, kind="ExternalOutput")

    NG = 6  # grams per batch: G1, G2, Sqc, Skc, Sqt, Skt

    with tile.TileContext(nc) as tc:
        with (
            tc.tile_pool(name="consts", bufs=1) as cpool,
            tc.tile_pool(name="xq", bufs=1) as xqpool,
            tc.tile_pool(name="xres", bufs=1) as xpool,
            tc.tile_pool(name="vres", bufs=1) as vpool,
            tc.tile_pool(name="qk", bufs=4) as qkpool,
            tc.tile_pool(name="work_ps", bufs=3, space="PSUM") as wps,
            tc.tile_pool(name="gram_ps", bufs=1, space="PSUM") as gps,
            tc.tile_pool(name="small", bufs=1) as spool,
            tc.tile_pool(name="obuf", bufs=1) as opool,
            tc.tile_pool(name="dram", bufs=1, space="DRAM") as dpool,
        ):
            # ---- weight AllGather: 1/8th slice per core -> full flat ----
            wsh_sb = cpool.tile([D, WSH // D], bf16, tag="wsh")
            nc.sync.dma_start(out=wsh_sb[:], in_=wsh[:])
            wag_in = dpool.tile([1, WSH], bf16, tag="wagin")
            wag_out = dpool.tile([1, WTOT], bf16, tag="wagout")
            nc.gpsimd.dma_start(out=wag_in[:], in_=wsh_sb[:])
            nc.gpsimd.collective_compute(
                "AllGather",
                mybir.AluOpType.bypass,
                replica_groups=[list(range(NC))],
                ins=[wag_in.opt()],
                outs=[wag_out.opt()],
            )

            # ---- unpack gathered weights into const tiles ----
            wqk_hi_sb = cpool.tile([D, 9, 2 * D], bf16, tag="wqkh")
            wqk_lo_sb = cpool.tile([D, 9, 2 * D], bf16, tag="wqkl")
            wv_hi_sb = cpool.tile([D, 9, D], bf16, tag="wvh")
            wv_lo_sb = cpool.tile([D, 9, D], bf16, tag="wvl")
            pct_sb = cpool.tile([D, D], bf16, tag="pct")
            ptt_sb = cpool.tile([D, D], bf16, tag="ptt")
            identb_sb = cpool.tile([D, D], bf16, tag="identb")
            wtiles = {"wqk_hi": wqk_hi_sb, "wqk_lo": wqk_lo_sb,
                      "wv_hi": wv_hi_sb, "wv_lo": wv_lo_sb,
                      "pct": pct_sb, "ptt": ptt_sb, "ident": identb_sb}
            off = 0
            for nm, n in WPACK:
                nc.gpsimd.dma_start(out=wtiles[nm][:], in_=wag_out[0, off:off + n])
                off += n
            ident_sb = cpool.tile([D, D], f32, tag="ident")
            nc.vector.tensor_copy(ident_sb[:], identb_sb[:])

            tempv_sb = cpool.tile([D, 1], f32, tag="tempv")
            biasv_sb = cpool.tile([D, 1], f32, tag="biasv")
            nc.sync.dma_start(out=tempv_sb[:], in_=tempvec[:])
            nc.sync.dma_start(out=biasv_sb[:], in_=biasvec[:])
            scq_sb = {}
            for s, ext in (("hi", sc_hi), ("lo", sc_lo)):
                for b in range(B):
                    t = cpool.tile([D, 1], f32, tag=f"sc{s}{b}")
                    nc.sync.dma_start(out=t[:], in_=ext[b])
                    scq_sb[(s, b)] = t

            # gram accumulation targets and per-batch v stores
            gram_cat = spool.tile([D, B * NG * D], f32, tag="gramcat")
            v_sb = {}   # (b, 'hi'/'lo') -> [D, RPC*W] bf16
            for b in range(B):
                for s in ("hi", "lo"):
                    v_sb[(b, s)] = vpool.tile([D, RPC * W], bf16,
                                              tag=f"v{b}{s}", name=f"v{b}{s}")

            xt = {}
            for b in range(B):
                # ---- load + dequantize this batch's input shards ----
                for s, ext in (("hi", x_hi), ("lo", x_lo)):
                    xq = xqpool.tile([D, HR, PW], i8, tag="xq")
                    nc.sync.dma_start(out=xq[:], in_=ext[b])
                    xd = xpool.tile([D, HR, PW], bf16, tag=f"x{s}")
                    nc.scalar.activation(xd[:], xq[:],
                                         mybir.ActivationFunctionType.Identity,
                                         bias=0.0, scale=scq_sb[(s, b)][:])
                    xt[(b, s)] = xd
                    del xq, xd

                # ---- pass A: q,k in [px, ch] + Gram/norm accumulation ----
                # paired layout sbp[:, g, :]: g=0 -> [q_c | k_t], g=1 -> [k_c | q_t]
                gA = gps.tile([D, 2 * D], f32, tag="gA", name=f"gA{b}")  # [Sqc | G1]
                gB = gps.tile([D, 2 * D], f32, tag="gB", name=f"gB{b}")  # [G2 | Sqt]
                gC = gps.tile([D, D], f32, tag="gC", name=f"gC{b}")      # Skt
                gD = gps.tile([D, D], f32, tag="gD", name=f"gD{b}")      # Skc

                def grams(sbp, first, last):
                    nc.tensor.matmul(gA[:], sbp[:, 0, 0:D], sbp[:, 0, :],
                                     start=first, stop=last)
                    nc.tensor.matmul(gB[:], sbp[:, 1, D:2 * D], sbp[:, 1, :],
                                     start=first, stop=last)
                    nc.tensor.matmul(gC[:], sbp[:, 0, D:2 * D], sbp[:, 0, D:2 * D],
                                     start=first, stop=last)
                    nc.tensor.matmul(gD[:], sbp[:, 1, 0:D], sbp[:, 1, 0:D],
                                     start=first, stop=last)

                prev = None
                for it in range(NT_A):
                    r = (it * PXT) // W          # output row 0..31
                    j = (it * PXT) % W           # 0 or 128
                    sbp = qkpool.tile([PXT, 2, 2 * D], bf16, tag="qksb")
                    for gi, (s, wsb) in enumerate((("hi", wqk_hi_sb),
                                                   ("lo", wqk_lo_sb))):
                        ps = wps.tile([PXT, 2 * D], f32, tag="apsum")
                        xs = xt[(b, s)]
                        for t in range(9):
                            dy, dx = t // 3, t % 3
                            lhsT = xs[:, r + dy, j + dx:j + dx + PXT]
                            nc.tensor.matmul(ps[:], lhsT, wsb[:, t, :],
                                             start=(t == 0), stop=(t == 8))
                        # hi [q_c|k_c] -> cols {0:96, 192:288}; lo [k_t|q_t] -> {96:192, 288:384}
                        nc.vector.tensor_copy(sbp[:, :, gi * D:(gi + 1) * D], ps[:])
                    if prev is not None:
                        grams(prev, prev_first, False)
                    prev_first = prev is None
                    prev = sbp
                grams(prev, False, True)

                for k, src in (("G1", gA[:, D:2 * D]), ("G2", gB[:, 0:D]),
                               ("Sqc", gA[:, 0:D]), ("Skc", gD[:]),
                               ("Sqt", gB[:, D:2 * D]), ("Skt", gC[:])):
                    gi = ("G1", "G2", "Sqc", "Skc", "Sqt", "Skt").index(k)
                    off = (b * NG + gi) * D
                    nc.vector.tensor_copy(gram_cat[:, off:off + D], src)

                # ---- pass B: v in [ch, px] ----
                for s, wsb in (("hi", wv_hi_sb), ("lo", wv_lo_sb)):
                    xs = xt[(b, s)]
                    for ck in range(NCHK):
                        r = ck * 2
                        ps = wps.tile([D, CHK], f32, tag="apsum")
                        for t in range(9):
                            dy, dx = t // 3, t % 3
                            rhs = xs[:, r + dy:r + dy + 2, dx:dx + W]
                            nc.tensor.matmul(ps[:], wsb[:, t, :], rhs,
                                             start=(t == 0), stop=(t == 8))
                        nc.vector.tensor_copy(
                            v_sb[(b, s)][:, ck * CHK:(ck + 1) * CHK], ps[:])

            # ---- AllReduce partial grams across the 8 cores ----
            ar_in = dpool.tile([D, B * NG * D], f32, tag="arin")
            ar_out = dpool.tile([D, B * NG * D], f32, tag="arout")
            nc.gpsimd.dma_start(out=ar_in[:], in_=gram_cat[:])
            nc.gpsimd.collective_compute(
                "AllReduce",
                mybir.AluOpType.add,
                replica_groups=[list(range(NC))],
                ins=[ar_in.opt()],
                outs=[ar_out.opt()],
            )
            gram_red = spool.tile([D, B * NG * D], f32, tag="gramred")
            nc.gpsimd.dma_start(out=gram_red[:], in_=ar_out[:])

            # ---- post-AR small compute per batch ----
            mt = {}  # (b, 'c'/'t') -> M^T tile [D, D] bf16
            for b in range(B):
                def gslice(gi):
                    off = (b * NG + gi) * D
                    return gram_red[:, off:off + D]
                G1, G2, Sqc, Skc, Sqt, Skt = [gslice(i) for i in range(NG)]

                rcol = {}
                for nm, S in (("qc", Sqc), ("kc", Skc), ("qt", Sqt), ("kt", Skt)):
                    tmp = spool.tile([D, D], f32, tag="dtmp")
                    nc.vector.tensor_tensor(out=tmp[:], in0=S, in1=ident_sb[:],
                                            op=mybir.AluOpType.mult)
                    dg = spool.tile([D, 1], f32, tag=f"d{nm}{b}")
                    nc.vector.tensor_reduce(out=dg[:], in_=tmp[:],
                                            axis=mybir.AxisListType.X,
                                            op=mybir.AluOpType.add)
                    sq = spool.tile([D, 1], f32, tag=f"sq{nm}{b}")
                    nc.scalar.sqrt(sq[:], dg[:])
                    rc = spool.tile([D, 1], f32, tag=f"rc{nm}{b}")
                    nc.vector.reciprocal(rc[:], sq[:])
                    rcol[nm] = rc
                # fold temperature into rq
                for nm in ("qc", "qt"):
                    nc.vector.tensor_tensor(out=rcol[nm][:], in0=rcol[nm][:],
                                            in1=tempv_sb[:],
                                            op=mybir.AluOpType.mult)

                # row-vector 1/||k|| via partition reduce of (S*I)
                rrow = {}
                for nm, S in (("kt", Skt), ("kc", Skc)):
                    tmp = spool.tile([D, D], f32, tag="dtmp")
                    nc.vector.tensor_tensor(out=tmp[:], in0=S, in1=ident_sb[:],
                                            op=mybir.AluOpType.mult)
                    drow = spool.tile([1, D], f32, tag=f"dr{nm}{b}")
                    nc.gpsimd.tensor_reduce(out=drow[:], in_=tmp[:],
                                            axis=mybir.AxisListType.C,
                                            op=mybir.AluOpType.add)
                    sqr = spool.tile([1, D], f32, tag=f"sqr{nm}{b}")
                    nc.scalar.sqrt(sqr[:], drow[:])
                    rr = spool.tile([1, D], f32, tag=f"rr{nm}{b}")
                    nc.vector.reciprocal(rr[:], sqr[:])
                    rb = spool.tile([D, D], f32, tag=f"rb{nm}{b}")
                    nc.gpsimd.partition_broadcast(rb[:], rr[:])
                    rrow[nm] = rb

                for attn_nm, G, rq, rkb, psb in (
                        ("c", G1, rcol["qc"], rrow["kt"], pct_sb),
                        ("t", G2, rcol["qt"], rrow["kc"], ptt_sb)):
                    L = spool.tile([D, D], f32, tag=f"L{attn_nm}{b}")
                    nc.vector.tensor_scalar(out=L[:], in0=G, scalar1=rq[:],
                                            scalar2=None,
                                            op0=mybir.AluOpType.mult)
                    nc.vector.tensor_tensor(out=L[:], in0=L[:], in1=rkb[:],
                                            op=mybir.AluOpType.mult)
                    A = spool.tile([D, D], bf16, tag=f"A{attn_nm}{b}")
                    nc.vector.memset(A[:], 0.0)
                    for h in range(HEADS):
                        p0 = 32 * h
                        blk = L[p0:p0 + 32, p0:p0 + 32]
                        nmax = spool.tile([32, 1], f32, tag=f"nm{attn_nm}{b}{h}")
                        nc.vector.tensor_reduce(out=nmax[:], in_=blk,
                                                axis=mybir.AxisListType.X,
                                                op=mybir.AluOpType.max,
                                                negate=True)
                        e = spool.tile([32, 32], f32, tag=f"e{attn_nm}{b}{h}")
                        nc.scalar.activation(e[:], blk,
                                             mybir.ActivationFunctionType.Exp,
                                             bias=nmax[:], scale=1.0)
                        ssum = spool.tile([32, 1], f32, tag=f"ss{attn_nm}{b}{h}")
                        nc.vector.tensor_reduce(out=ssum[:], in_=e[:],
                                                axis=mybir.AxisListType.X,
                                                op=mybir.AluOpType.add)
                        rs = spool.tile([32, 1], f32, tag=f"rs{attn_nm}{b}{h}")
                        nc.vector.reciprocal(rs[:], ssum[:])
                        nc.vector.tensor_scalar(out=A[p0:p0 + 32, p0:p0 + 32],
                                                in0=e[:], scalar1=rs[:],
                                                scalar2=None,
                                                op0=mybir.AluOpType.mult)
                    # M^T = A(lhsT) . P^T  -> [d, o]
                    mps = wps.tile([D, D], f32, tag="apsum")
                    nc.tensor.matmul(mps[:], A[:], psb[:], start=True, stop=True)
                    msb = spool.tile([D, D], bf16, tag=f"m{attn_nm}{b}")
                    nc.vector.tensor_copy(msb[:], mps[:])
                    mt[(b, attn_nm)] = msb

            # ---- final: out = M_cT^T @ v_t + M_tT^T @ v_c + bias ----
            # Stage per-batch output in SBUF (bf16), track per-channel
            # absmax, then quantize to int8 with per-channel scale.
            for b in range(B):
                ob = opool.tile([D, RPC * W], bf16, tag="ob", name=f"ob{b}")
                amax = spool.tile([D, 1], f32, tag=f"amax{b}")
                for ck in range(NCHK):
                    ps = wps.tile([D, CHK], f32, tag="apsum")
                    sl = slice(ck * CHK, (ck + 1) * CHK)
                    nc.tensor.matmul(ps[:], mt[(b, "c")][:], v_sb[(b, "lo")][:, sl],
                                     start=True, stop=False)
                    nc.tensor.matmul(ps[:], mt[(b, "t")][:], v_sb[(b, "hi")][:, sl],
                                     start=False, stop=True)
                    nc.scalar.activation(ob[:, sl], ps[:],
                                         mybir.ActivationFunctionType.Identity,
                                         bias=biasv_sb[:], scale=1.0)
                    oabs = spool.tile([D, CHK], f32, tag="oabs")
                    nc.scalar.activation(oabs[:], ps[:],
                                         mybir.ActivationFunctionType.Abs,
                                         bias=biasv_sb[:], scale=1.0)
                    cmax = spool.tile([D, 1], f32, tag=f"cmax{b}")
                    nc.vector.tensor_reduce(out=cmax[:], in_=oabs[:],
                                            axis=mybir.AxisListType.X,
                                            op=mybir.AluOpType.max)
                    if ck == 0:
                        nc.vector.tensor_copy(amax[:], cmax[:])
                    else:
                        nc.vector.tensor_tensor(out=amax[:], in0=amax[:],
                                                in1=cmax[:],
                                                op=mybir.AluOpType.max)
                # scale = amax/127 (host dequant), rscale = 127/amax
                scl = spool.tile([D, 1], f32, tag=f"scl{b}")
                nc.vector.tensor_scalar(out=scl[:], in0=amax[:],
                                        scalar1=1.0 / 127.0, scalar2=None,
                                        op0=mybir.AluOpType.mult)
                nc.sync.dma_start(out=out_sc[b], in_=scl[:])
                rsc = spool.tile([D, 1], f32, tag=f"rsc{b}")
                nc.vector.reciprocal(rsc[:], scl[:])
                oq = opool.tile([D, RPC * W], i8, tag="oq", name=f"oq{b}")
                nc.scalar.activation(oq[:], ob[:],
                                     mybir.ActivationFunctionType.Identity,
                                     bias=0.0, scale=rsc[:])
                nc.sync.dma_start(out=out_i8[b], in_=oq[:])

    nc.compile()
    return nc


def _get_nc():
    if "nc" not in _CACHE:
        from concourse import bass, bacc, tile, mybir
        _CACHE["mods"] = (bass, bacc, tile, mybir)
        _CACHE["nc"] = _build(_CACHE["mods"])
    return _CACHE["nc"]


def _quant_in(x):
    """x: [B,D,H,W] f32 -> (int8 padded shards per core, scales [B,D,1] f32)."""
    x = np.asarray(x, np.float32)
    sc = np.abs(x).max(axis=(2, 3), keepdims=True) / 127.0  # [B,D,1,1]
    xq = np.clip(np.round(x / sc), -127, 127).astype(np.int8)
    xp = np.zeros((B, D, H + 2, PW), np.int8)
    xp[:, :, 1:H + 1, 1:W + 1] = xq
    sh = []
    for c in range(NC):
        r0 = c * RPC
        sh.append(np.ascontiguousarray(xp[:, :, r0:r0 + HR, :]))
    return sh, np.ascontiguousarray(sc[:, :, :, 0])


def _prep_inputs(low, high, temperature, qc_w, qdw_c_w, kvc_w, kvdw_c_w,
                 qt_w, qdw_t_w, kvt_w, kvdw_t_w, po_c_w, po_t_w,
                 concat_w, concat_b):
    """Host-side weight folding + input shard/pad/quant. Returns in_maps."""
    W3 = {
        "q_hi": _fold3x3(qc_w, qdw_c_w),
        "k_hi": _fold3x3(kvc_w[:96], kvdw_c_w[:96]),
        "v_hi": _fold3x3(kvc_w[96:], kvdw_c_w[96:]),
        "q_lo": _fold3x3(qt_w, qdw_t_w),
        "k_lo": _fold3x3(kvt_w[:96], kvdw_t_w[:96]),
        "v_lo": _fold3x3(kvt_w[96:], kvdw_t_w[96:]),
    }
    wqk_hi = _bf16(np.concatenate([W3["q_hi"], W3["k_hi"]], axis=2))  # [9,96,192]
    wqk_lo = _bf16(np.concatenate([W3["k_lo"], W3["q_lo"]], axis=2))
    wv_hi = _bf16(W3["v_hi"])
    wv_lo = _bf16(W3["v_lo"])
    # device layout [D(ci), 9, O]
    wqk_hi = np.ascontiguousarray(wqk_hi.transpose(1, 0, 2))
    wqk_lo = np.ascontiguousarray(wqk_lo.transpose(1, 0, 2))
    wv_hi = np.ascontiguousarray(wv_hi.transpose(1, 0, 2))
    wv_lo = np.ascontiguousarray(wv_lo.transpose(1, 0, 2))
    P_c = concat_w[:, :96] @ po_c_w
    P_t = concat_w[:, 96:] @ po_t_w
    pct = _bf16(P_c.T)
    ptt = _bf16(P_t.T)
    ident = _bf16(np.eye(D, dtype=np.float32))
    tempv = np.repeat(np.asarray(temperature, np.float32).reshape(3), 32)[:, None]
    biasv = np.asarray(concat_b, np.float32)[:, None]

    # pack all bf16 weights into one flat buffer, split 8 ways
    wflat = np.concatenate([
        wqk_hi.ravel(), wqk_lo.ravel(), wv_hi.ravel(), wv_lo.ravel(),
        pct.ravel(), ptt.ravel(), ident.ravel()])
    assert wflat.size == WTOT
    wshards = [np.ascontiguousarray(wflat[c * WSH:(c + 1) * WSH].reshape(1, WSH))
               for c in range(NC)]

    lo_sh, lo_sc = _quant_in(low)
    hi_sh, hi_sc = _quant_in(high)

    in_maps = []
    for c in range(NC):
        in_maps.append({
            "x_hi": hi_sh[c], "x_lo": lo_sh[c],
            "sc_hi": hi_sc, "sc_lo": lo_sc,
            "wsh": wshards[c],
            "tempvec": tempv, "biasvec": biasv,
        })
    return in_maps


def run(trace=False, in_maps=None, **inputs):
    import time as _time
    from concourse.bass_utils import run_bass_kernel_spmd
    nc = _get_nc()
    if in_maps is None:
        in_maps = _prep_inputs(**inputs)
    t0 = _time.time()
    res = run_bass_kernel_spmd(nc, in_maps, list(range(NC)), trace=trace)
    res.dispatch_wall_s = _time.time() - t0
    res.in_maps = in_maps
    out = np.empty((B, D, H, W), np.float32)
    for c in range(NC):
        oi = res.results[c]["out_i8"].astype(np.float32)
        oscl = res.results[c]["out_sc"].reshape(B, D, 1, 1)
        out[:, :, c * RPC:(c + 1) * RPC, :] = oi * oscl
    return out, res


def kernel(**inputs):
    out, _ = run(trace=False, **inputs)
    return out
